# revision 4
# baseline (speedup 1.0000x reference)
"""KMeans label-assignment kernel for Trainium2 (Bass/Tile), 8 NeuronCores.

Problem: x [500000, 64] f32, centroids [512, 64] f32 -> labels [500000] int32
labels[n] = argmin_k ||x[n] - c[k]||^2  == argmax_k (x[n].c[k] - ||c[k]||^2/2)

Strategy (data parallel over N):
  - Host: shard x row-wise across 8 cores; per shard build transposed,
    fp16 hi/lo-split inputs xT_hi/xT_lo [65, NP] (row 64 = ones in hi),
    and the shared cT_hi/cT_lo [65, 512] (row 64 = -||c_k||^2/2 split).
  - Device per tile of 128 rows: three fp16 matmuls accumulate
    scores = x.c - ||c||^2/2 in fp32 PSUM [128, 512]
    (hi.hi + hi.lo + lo.hi; the lo.lo term is ~2^-22, dropped).
  - One fused custom-DVE op per tile computes the argmax in a single
    512-element pass: stream scores reversed, running max via scan,
    emit Idx where the element equals the running max, MAX-accumulate
    -> last leader position j* ; label = 511 - j*. Exact, including
    first-occurrence tie-break semantics.
  - accum_out writes straight into the label plane; DMA out as [128, T]
    f32; host converts: label = 511 - round(v).
"""

import numpy as np
import sys

sys.path.insert(0, "/opt/trn_rl_repo")

N, D, K = 500_000, 64, 512
NCORES = 8
NS = N // NCORES            # 62500 rows per core
TILE = 128                  # rows per matmul tile (PSUM partitions)
G = 8                       # tiles per DMA chunk / label group
T = 496                     # padded tiles per core (62 chunks of 8)
NP = T * TILE               # padded rows per core = 63488
NCHUNK = T // G             # 62

_cache = {}


def _register_argmax_op():
    """Register the fused single-pass argmax custom-DVE op at runtime."""
    import concourse.dve_ops as dve_ops
    from concourse.dve_ops import DveOp, OPS, CUSTOM_DVE_SPECS, _SUB_OPCODE_FOR_NAME
    from concourse.dve_spec import (
        Spec, Src0, Idx, MaxNeg, eq, select, scan, lower, AluOp, _has_src1,
    )
    from concourse.dve_uop import DveOpSpec

    name = "ARGMAX_REV_ANT"
    if name in _SUB_OPCODE_FOR_NAME:
        return next(op for op in OPS if op.name == name)

    r = scan(AluOp.MAX, Src0)
    spec = Spec(body=select(eq(Src0, r), Idx, MaxNeg), accum=AluOp.MAX)
    opcode = dve_ops._CUSTOM_DVE_ROW_BASE + len(OPS)
    assert opcode < 0x20
    _SUB_OPCODE_FOR_NAME[name] = opcode
    shas = {}
    for ver in ("v3",):  # TRN2
        s = DveOpSpec(
            name=name, opcode=opcode, uops=lower(spec, ver=ver),
            rd1_en=_has_src1(spec),
        )
        shas[ver] = s.sha(ver)
    op = DveOp(name, spec, subdim=False, uops_sha=shas)
    OPS.append(op)
    CUSTOM_DVE_SPECS[name] = spec
    return op


def _build():
    import concourse.bacc as bacc
    import concourse.tile as tile
    from concourse import mybir

    f16 = mybir.dt.float16
    f32 = mybir.dt.float32

    argmax_op = _register_argmax_op()

    nc = bacc.Bacc("TRN2", target_bir_lowering=False)
    xh = nc.dram_tensor("xh", [D + 1, NP], f16, kind="ExternalInput")
    xl = nc.dram_tensor("xl", [D + 1, NP], f16, kind="ExternalInput")
    ch = nc.dram_tensor("ch", [D + 1, K], f16, kind="ExternalInput")
    cl = nc.dram_tensor("cl", [D + 1, K], f16, kind="ExternalInput")
    lab = nc.dram_tensor("lab", [TILE, T], f32, kind="ExternalOutput")

    with tile.TileContext(nc) as tc:
        with tc.tile_pool(name="cpool", bufs=1) as cpool, \
             tc.tile_pool(name="xpool", bufs=3) as xpool, \
             tc.tile_pool(name="ppool", bufs=8, space="PSUM") as ppool, \
             tc.tile_pool(name="spool", bufs=2) as spool, \
             tc.tile_pool(name="lpool", bufs=2) as lpool:
            chtile = cpool.tile([D + 1, K], f16, tag="ch")
            cltile = cpool.tile([D + 1, K], f16, tag="cl")
            nc.sync.dma_start(chtile[:], ch[:, :])
            nc.sync.dma_start(cltile[:], cl[:, :])
            for chk in range(NCHUNK):
                sl = slice(chk * G * TILE, (chk + 1) * G * TILE)
                xhtile = xpool.tile([D + 1, G * TILE], f16, tag="xh")
                nc.sync.dma_start(xhtile[:], xh[:, sl])
                xltile = xpool.tile([D + 1, G * TILE], f16, tag="xl")
                nc.sync.dma_start(xltile[:], xl[:, sl])
                ltile = lpool.tile([TILE, G], f32)
                for j in range(G):
                    tsl = slice(j * TILE, (j + 1) * TILE)
                    ps = ppool.tile([TILE, K], f32)
                    nc.tensor.matmul(
                        ps[:], lhsT=xhtile[:, tsl], rhs=chtile[:],
                        start=True, stop=False,
                    )
                    nc.tensor.matmul(
                        ps[:], lhsT=xhtile[:, tsl], rhs=cltile[:],
                        start=False, stop=False,
                    )
                    nc.tensor.matmul(
                        ps[:], lhsT=xltile[:, tsl], rhs=chtile[:],
                        start=False, stop=True,
                    )
                    trash = spool.tile([TILE, K], f32, tag="trash")
                    nc.vector._custom_dve(
                        argmax_op,
                        out=trash[:],
                        in0=ps[:, ::-1],
                        accum_out=ltile[:, j:j + 1],
                    )
                nc.sync.dma_start(lab[:, chk * G:(chk + 1) * G], ltile[:])
    nc.compile()
    return nc


def _get_nc():
    if "nc" not in _cache:
        _cache["nc"] = _build()
    return _cache["nc"]


def _split16(a):
    hi = a.astype(np.float16)
    lo = (a - hi.astype(np.float32)).astype(np.float16)
    return hi, lo


def make_in_maps(x, c):
    x = np.ascontiguousarray(np.asarray(x, dtype=np.float32))
    c = np.ascontiguousarray(np.asarray(c, dtype=np.float32))

    c2 = (c * c).sum(axis=1, dtype=np.float32)
    ct_aug = np.empty((D + 1, K), dtype=np.float32)
    ct_aug[:D] = c.T
    ct_aug[D] = -0.5 * c2
    ch_, cl_ = _split16(ct_aug)

    in_maps = []
    for i in range(NCORES):
        xs = x[i * NS:(i + 1) * NS]
        xt_aug = np.zeros((D + 1, NP), dtype=np.float32)
        xt_aug[:D, :NS] = xs.T
        xt_aug[D, :] = 1.0
        xh_, xl_ = _split16(xt_aug)
        in_maps.append({"xh": xh_, "xl": xl_, "ch": ch_, "cl": cl_})
    return in_maps


def kernel(x, centroids):
    from concourse.bass_utils import run_bass_kernel_spmd

    in_maps = make_in_maps(x, centroids)
    nc = _get_nc()
    res = run_bass_kernel_spmd(nc, in_maps, core_ids=list(range(NCORES)))
    out = np.empty(N, dtype=np.int32)
    for i in range(NCORES):
        plane = res.results[i]["lab"]          # [128, T] float32
        lab_i = (K - 1) - np.rint(plane.T.reshape(-1)[:NS]).astype(np.int32)
        out[i * NS:(i + 1) * NS] = lab_i
    return out


# revision 7
# speedup vs baseline: 1.2284x; 1.2284x over previous
"""KMeans label-assignment kernel for Trainium2 (Bass/Tile), 8 NeuronCores.

Problem: x [500000, 64] f32, centroids [512, 64] f32 -> labels [500000] int32
labels[n] = argmin_k ||x[n] - c[k]||^2  == argmax_k (x[n].c[k] - ||c[k]||^2/2)

Strategy (data parallel over N):
  - Host: shard x row-wise across 8 cores; per shard build transposed,
    fp16 hi/lo-split inputs xT_hi/xT_lo [65, NP] (row 64 = ones in hi),
    and the shared cT_hi/cT_lo [65, 512] (row 64 = -||c_k||^2/2 split).
  - Device per tile of 128 rows: three fp16 matmuls accumulate
    scores = x.c - ||c||^2/2 in fp32 PSUM [128, 512]
    (hi.hi + hi.lo + lo.hi; the lo.lo term is ~2^-22, dropped).
  - One fused custom-DVE op per tile computes the argmax in a single
    512-element pass: stream scores reversed, running max via scan,
    emit Idx where the element equals the running max, MAX-accumulate
    -> last leader position j* ; label = 511 - j*. Exact, including
    first-occurrence tie-break semantics.
  - accum_out writes straight into the label plane; DMA out as [128, T]
    f32; host converts: label = 511 - round(v).
"""

import numpy as np
import sys

sys.path.insert(0, "/opt/trn_rl_repo")

N, D, K = 500_000, 64, 512
NCORES = 8
NS = N // NCORES            # 62500 rows per core
TILE = 128                  # rows per matmul tile (PSUM partitions)
G = 8                       # tiles per DMA chunk / label group
T = 496                     # padded tiles per core (62 chunks of 8)
NP = T * TILE               # padded rows per core = 63488
NCHUNK = T // G             # 62

_cache = {}


def _register_argmax_op():
    """Register the fused single-pass argmax custom-DVE op at runtime."""
    import concourse.dve_ops as dve_ops
    from concourse.dve_ops import DveOp, OPS, CUSTOM_DVE_SPECS, _SUB_OPCODE_FOR_NAME
    from concourse.dve_spec import (
        Spec, Src0, Idx, MaxNeg, eq, select, scan, lower, AluOp, _has_src1,
    )
    from concourse.dve_uop import DveOpSpec

    name = "ARGMAX_REV_ANT"
    if name in _SUB_OPCODE_FOR_NAME:
        return next(op for op in OPS if op.name == name)

    r = scan(AluOp.MAX, Src0)
    spec = Spec(body=select(eq(Src0, r), Idx, MaxNeg), accum=AluOp.MAX)
    opcode = dve_ops._CUSTOM_DVE_ROW_BASE + len(OPS)
    assert opcode < 0x20
    _SUB_OPCODE_FOR_NAME[name] = opcode
    shas = {}
    for ver in ("v3",):  # TRN2
        s = DveOpSpec(
            name=name, opcode=opcode, uops=lower(spec, ver=ver),
            rd1_en=_has_src1(spec),
        )
        shas[ver] = s.sha(ver)
    op = DveOp(name, spec, subdim=False, uops_sha=shas)
    OPS.append(op)
    CUSTOM_DVE_SPECS[name] = spec
    return op


def _build():
    import concourse.bacc as bacc
    import concourse.tile as tile
    from concourse import mybir

    f16 = mybir.dt.float16
    f32 = mybir.dt.float32

    argmax_op = _register_argmax_op()

    nc = bacc.Bacc("TRN2", target_bir_lowering=False)
    x1 = nc.dram_tensor("x1", [127, NP], f16, kind="ExternalInput")
    x2 = nc.dram_tensor("x2", [67, NP], f16, kind="ExternalInput")
    m1 = nc.dram_tensor("m1", [127, K], f16, kind="ExternalInput")
    m2 = nc.dram_tensor("m2", [67, K], f16, kind="ExternalInput")
    lab = nc.dram_tensor("lab", [TILE, T], f32, kind="ExternalOutput")

    with tile.TileContext(nc) as tc:
        with tc.tile_pool(name="cpool", bufs=1) as cpool, \
             tc.tile_pool(name="xpool", bufs=3) as xpool, \
             tc.tile_pool(name="ppool", bufs=8, space="PSUM") as ppool, \
             tc.tile_pool(name="spool", bufs=2) as spool, \
             tc.tile_pool(name="lpool", bufs=2) as lpool:
            m1tile = cpool.tile([127, K], f16, tag="m1")
            m2tile = cpool.tile([67, K], f16, tag="m2")
            nc.sync.dma_start(m1tile[:], m1[:, :])
            nc.sync.dma_start(m2tile[:], m2[:, :])
            for chk in range(NCHUNK):
                sl = slice(chk * G * TILE, (chk + 1) * G * TILE)
                x1tile = xpool.tile([127, G * TILE], f16, tag="x1")
                nc.sync.dma_start(x1tile[:], x1[:, sl])
                x2tile = xpool.tile([67, G * TILE], f16, tag="x2")
                nc.sync.dma_start(x2tile[:], x2[:, sl])
                ltile = lpool.tile([TILE, G], f32)
                for j in range(G):
                    tsl = slice(j * TILE, (j + 1) * TILE)
                    ps = ppool.tile([TILE, K], f32)
                    nc.tensor.matmul(
                        ps[:], lhsT=x1tile[:, tsl], rhs=m1tile[:],
                        start=True, stop=False,
                    )
                    nc.tensor.matmul(
                        ps[:], lhsT=x2tile[:, tsl], rhs=m2tile[:],
                        start=False, stop=True,
                    )
                    trash = spool.tile([TILE, K], f32, tag="trash")
                    nc.vector._custom_dve(
                        argmax_op,
                        out=trash[:],
                        in0=ps[:, ::-1],
                        accum_out=ltile[:, j:j + 1],
                    )
                nc.sync.dma_start(lab[:, chk * G:(chk + 1) * G], ltile[:])
    nc.compile()
    return nc


def _get_nc():
    if "nc" not in _cache:
        _cache["nc"] = _build()
    return _cache["nc"]


def _split16(a):
    hi = a.astype(np.float16)
    lo = (a - hi.astype(np.float32)).astype(np.float16)
    return hi, lo


def make_in_maps(x, c):
    x = np.ascontiguousarray(np.asarray(x, dtype=np.float32))
    c = np.ascontiguousarray(np.asarray(c, dtype=np.float32))

    c2 = (c * c).sum(axis=1, dtype=np.float32)
    ct_aug = np.empty((D + 1, K), dtype=np.float32)
    ct_aug[:D] = c.T
    ct_aug[D] = -0.5 * c2
    ch_, cl_ = _split16(ct_aug)
    # pass-1 moving: [ch rows 0-63 ; ch rows 0-62]  (pairs xh, xl[0:63])
    m1_ = np.ascontiguousarray(np.concatenate([ch_[0:D], ch_[0:D - 1]], 0))
    # pass-2 moving: [cl rows 0-63 ; bias_hi ; ch row 63 ; bias_lo]
    m2_ = np.ascontiguousarray(np.concatenate(
        [cl_[0:D], ch_[D:D + 1], ch_[D - 1:D], cl_[D:D + 1]], 0))

    in_maps = []
    ones = np.ones((1, NP), dtype=np.float16)
    for i in range(NCORES):
        xs = x[i * NS:(i + 1) * NS]
        xt = np.zeros((D, NP), dtype=np.float32)
        xt[:, :NS] = xs.T
        xh_, xl_ = _split16(xt)
        # pass-1 stationary: [xh rows 0-63 ; xl rows 0-62]
        x1_ = np.ascontiguousarray(np.concatenate([xh_, xl_[0:D - 1]], 0))
        # pass-2 stationary: [xh rows 0-63 ; ones ; xl row 63 ; ones]
        x2_ = np.ascontiguousarray(np.concatenate(
            [xh_, ones, xl_[D - 1:D], ones], 0))
        in_maps.append({"x1": x1_, "x2": x2_, "m1": m1_, "m2": m2_})
    return in_maps


def kernel(x, centroids):
    from concourse.bass_utils import run_bass_kernel_spmd

    in_maps = make_in_maps(x, centroids)
    nc = _get_nc()
    res = run_bass_kernel_spmd(nc, in_maps, core_ids=list(range(NCORES)))
    out = np.empty(N, dtype=np.int32)
    for i in range(NCORES):
        plane = res.results[i]["lab"]          # [128, T] float32
        lab_i = (K - 1) - np.rint(plane.T.reshape(-1)[:NS]).astype(np.int32)
        out[i * NS:(i + 1) * NS] = lab_i
    return out
